# revision 41
# baseline (speedup 1.0000x reference)
"""Trainium2 Bass kernel for nn_Attention (dense transformer block).

Strategy: data-parallel over batch across 8 NeuronCores (8 batches/core).
Per core, per batch (N=256 tokens, 16 heads, dim_head=32):
  - qkv projection: q,k computed TRANSPOSED (qkT [j, n], weight-stationary),
    v computed untransposed (x^T-stationary) -> v [n, j] so the attn@v matmul
    needs no on-chip transposes at all.
  - dots^T[m, n] = k_h^T.T-stationary @ q_h^T streaming, K=32, 4 heads packed
    into the 4 PE row-groups via a zero-padded q staging buffer (row-tiled
    matmuls at nonzero tile_position fault this HW/compiler combo).
  - softmax without max-subtraction (|dots| <~ 1.5 by construction) and with
    normalization deferred: attn_unnorm = exp(dots^T) * exp(bias^T) (exp on
    ACT reading PSUM directly, bias multiply split DVE/GPSIMD in bf16).
  - attn@v: out_h^T[d, n] = v_h[m, d]-stationary @ attn^T streaming, 4 heads
    packed into PE col-groups (emitted back-to-back so the col-group matmuls
    execute concurrently in disjoint column groups of the PE array); a
    parallel ones[m,32]-stationary matmul computes the softmax denominators
    as a 32-row broadcast.
  - out projection with b_out folded in as a K=1 matmul row; PSUM -> DRAM DMA.
  - depth-2 software pipeline: front(b) [qkv proj + q0 staging] runs with
    dots(b-1) and attn@v/outproj(b-2) interleaved per head-group, hiding the
    q0-staging DMA latency and the exp/bias-mul latency under PE work.
  - j-tile-progressive wqkv loads (j-tile-major DRAM layout) let the first
    projection matmul start ~10us earlier at kernel startup.
All matmuls in bf16 (fp32 PSUM accumulation); rel-err vs fp32 reference ~1e-3.
"""

import os
import sys

import numpy as np

if "/opt/trn_rl_repo" not in sys.path:
    sys.path.insert(0, "/opt/trn_rl_repo")

import ml_dtypes  # noqa: E402

from concourse import bacc, mybir  # noqa: E402
from concourse.tile import TileContext  # noqa: E402
from concourse.bass_utils import run_bass_kernel_spmd  # noqa: E402

BF16 = mybir.dt.bfloat16
F32 = mybir.dt.float32
NPBF16 = ml_dtypes.bfloat16

B, N, INP, OUP, H, D = 64, 256, 512, 512, 16, 32
NCORES = 8
BL = B // NCORES  # batches per core
SCALE = D ** -0.5

_CACHE = {}


def _relative_index(ih: int, iw: int) -> np.ndarray:
    yy, xx = np.meshgrid(np.arange(ih), np.arange(iw), indexing="ij")
    coords = np.stack([yy.ravel(), xx.ravel()])
    rel = coords[:, :, None] - coords[:, None, :]
    rel[0] += ih - 1
    rel[1] += iw - 1
    rel[0] *= 2 * iw - 1
    return rel.sum(0).ravel()


DEFAULT_OPTS = {
    "bias_dve_mod": 3,      # (2g+mt) % 8 < this -> DVE, else GPSIMD
    "hoist": False,         # LDW hoisting: measured neutral, keep off
    "interleave": True,     # software-pipeline attn@v(b-1) under dots(b)
    "q0_merged_dma": True,  # 4 merged q0 DMAs vs 16 per-head
    "depth": 2,             # software-pipeline depth (1 or 2)
    "stt_div": False,       # DVE divide: rejected by walrus codegen on TRN2
    "vt_on_act": False,     # ACT-queue FIFO delays vt behind exps: keep on DVE
    "pd_bufs": 2,
    "unified_pd": False,
    "pod_bufs": 2,
    "xpool_bufs": 3,
    "qkv_bufs": 3,
    "attn_bufs": 2,
    "evac_chunks": 2,
    "exp_split": False,     # 512-col exp/mul halves: measured slower
    "small_bufs": 4,
    "q0_bufs": 2,
}


def _hoist_ldw_bundles(nc, bundles):
    """Reorder each bundle's [LDW MM LDW MM ...] pairs to [LDW... MM...].

    Only the PE-stream relative order changes; LDWEIGHTS carry no semaphore
    updates, so every cross-engine wait threshold keeps its meaning.  The
    instruction objects are written back into the same block slots, leaving
    other engines' instructions untouched.
    """
    fn = nc.m.functions[0]
    n_done = 0
    for blk in fn.blocks:
        ilist = blk.instructions
        name2idx = {inst.name: k for k, inst in enumerate(ilist)}
        for bundle in bundles:
            idxs = [name2idx.get(n) for n in bundle]
            if any(i is None for i in idxs):
                continue
            slots = []
            ok = True
            for i in idxs:
                if i == 0 or not isinstance(ilist[i - 1], mybir.InstLdweights):
                    ok = False
                    break
                slots.extend([i - 1, i])
            if not ok:
                continue
            # a foreign PE instruction inside the span would have its
            # LDWEIGHTS clobber the hoisted col strips -> only hoist
            # bundles whose pair slots are contiguous on the PE stream
            slot_set = set(slots)
            for k in range(min(slots), max(slots) + 1):
                if k not in slot_set and ilist[k].engine == mybir.EngineType.PE:
                    ok = False
                    break
            if not ok:
                continue
            slots_sorted = sorted(slots)
            ldws = [ilist[i - 1] for i in idxs]
            mms = [ilist[i] for i in idxs]
            new_order = ldws + mms
            for slot, inst in zip(slots_sorted, new_order):
                ilist[slot] = inst
            n_done += 1
    if os.environ.get("BASS_DEBUG_HOIST"):
        print(f"hoisted {n_done}/{len(bundles)} LDW bundles")
    return n_done


def _build(bl: int, repeats: int = 1, opts: dict | None = None):
    o = dict(DEFAULT_OPTS)
    if opts:
        o.update(opts)
    nc = bacc.Bacc(None, target_bir_lowering=False)

    xT = nc.declare_dram_parameter("xT", [bl, 128, 4, 256], BF16, isOutput=False)
    # wqkv[p, jt, it, jj] = w_qkv[it*128+p, jt*128+jj] -- j-tile-major so the
    # per-j-tile startup loads are contiguous
    wqkv = nc.declare_dram_parameter("wqkv", [128, 12, 4, 128], BF16, isOutput=False)
    w2t = nc.declare_dram_parameter("w2t", [128, 4, 512], BF16, isOutput=False)
    ebT = nc.declare_dram_parameter("ebT", [128, 2, 4096], BF16, isOutput=False)
    bout = nc.declare_dram_parameter("bout", [1, 512], F32, isOutput=False)
    boutb = nc.declare_dram_parameter("boutb", [1, 512], BF16, isOutput=False)
    y = nc.declare_dram_parameter("y", [bl, 2, 128, 512], F32, isOutput=True)

    EXP = mybir.ActivationFunctionType.Exp
    bundles = []  # lists of InstMatmult names whose LDWs get hoisted

    with TileContext(nc) as tc:
        with (
            tc.tile_pool(name="consts", bufs=1) as consts,
            tc.tile_pool(name="xpool", bufs=o["xpool_bufs"]) as xpool,
            tc.tile_pool(name="qkvpool", bufs=o["qkv_bufs"]) as qkvpool,
            tc.tile_pool(name="attnpool", bufs=o["attn_bufs"]) as attnpool,
            tc.tile_pool(name="small", bufs=o["small_bufs"]) as small,
            tc.tile_pool(name="pbig", bufs=o["pd_bufs"], space="PSUM") as pbig,
            tc.tile_pool(name="pod", bufs=o["pod_bufs"], space="PSUM") as pod,
        ):
            # constant loads spread across engine DMA queues so the first
            # batch's x tile (sync queue) isn't stuck behind them
            # j-tile-progressive weight loads: the first projection matmul
            # group needs only x and wq j-tile 0, so it can start as soon as
            # those land instead of waiting for the full 1.5 MB of wqkv.
            xt_pre = xpool.tile([128, 4, 256], BF16, tag="xt", name="xt")
            nc.sync.dma_start(xt_pre[:, 0:2, :], xT[0, :, 0:2, :])
            nc.scalar.dma_start(xt_pre[:, 2:4, :], xT[0, :, 2:4, :])
            wq_sb = consts.tile([128, 12, 4, 128], BF16)
            # jt0 on the SWDGE queue: it starts flowing earliest and the
            # bias table behind it isn't needed until the first bias-mul
            nc.gpsimd.dma_start(wq_sb[:, 0], wqkv[:, 0])
            for jt in range(1, 4):
                eng = nc.scalar if jt % 2 == 0 else nc.sync
                eng.dma_start(wq_sb[:, jt], wqkv[:, jt])
            nc.sync.dma_start(wq_sb[:, 4:8], wqkv[:, 4:8])
            nc.scalar.dma_start(wq_sb[:, 8:12], wqkv[:, 8:12])
            eb_sb = consts.tile([128, 2, 4096], BF16)
            nc.gpsimd.dma_start(eb_sb[:], ebT[:])
            w2_sb = consts.tile([128, 4, 512], BF16)
            nc.scalar.dma_start(w2_sb[:], w2t[:])
            bout_bc = consts.tile([128, 512], F32)
            nc.scalar.dma_start(bout_bc[:], bout[:].to_broadcast((128, 512)))
            bo_sb = consts.tile([1, 512], BF16)
            nc.scalar.dma_start(bo_sb[:], boutb[:])
            ones32 = consts.tile([128, 32], BF16)
            nc.vector.memset(ones32[:], 1.0)
            ones1 = consts.tile([1, 128], BF16)
            nc.vector.memset(ones1[:], 1.0)
            # zero-padded q staging: q0[p, h, n] nonzero only for
            # p in [32*(h%4), 32*(h%4)+32); the zero rows are written once
            # and never touched again (per-batch DMAs overwrite only the
            # nonzero rows), so the cross-head terms of the full-K dots
            # matmuls vanish. Two buffers, alternated by batch parity.
            q0 = []
            for i in range(o["q0_bufs"]):
                t = consts.tile([128, 16, 256], BF16, name=f"q0_{i}")
                nc.vector.memset(t[:], 0.0)
                q0.append(t)

            def emit_front(rep, b):
                """qkv projection + dots + exp + bias-mul for batch b."""
                if rep == 0 and b == 0:
                    xt = xt_pre
                else:
                    xt = xpool.tile([128, 4, 256], BF16, tag="xt", name="xt")
                    nc.sync.dma_start(xt[:], xT[b])

                qkT = qkvpool.tile([128, 2048], BF16, tag="qkT")
                vt = qkvpool.tile([128, 2, 512], BF16, tag="vt")

                # q,k projection (transposed): out[j, n] over j-tiles 0..7
                for half in range(2):
                    pqk = pbig.tile(
                        [128, 1024], F32, name="pqk",
                        **({"tag": "pd"} if o["unified_pd"] else {"tag": "pqk", "bufs": 1}),
                    )
                    for jq in range(4):
                        jt = half * 4 + jq
                        for it in range(4):
                            nc.tensor.matmul(
                                pqk[:, jq * 256 : (jq + 1) * 256],
                                lhsT=wq_sb[:, jt, it, :],
                                rhs=xt[:, it, :],
                                start=(it == 0),
                                stop=(it == 3),
                            )
                    if o["evac_chunks"] == 1:
                        nc.vector.tensor_copy(
                            out=qkT[:, half * 1024 : (half + 1) * 1024], in_=pqk[:]
                        )
                    else:
                        ch = 1024 // o["evac_chunks"]
                        for ci in range(o["evac_chunks"]):
                            nc.vector.tensor_copy(
                                out=qkT[
                                    :,
                                    half * 1024 + ci * ch : half * 1024
                                    + (ci + 1) * ch,
                                ],
                                in_=pqk[:, ci * ch : (ci + 1) * ch],
                            )

                # v projection (untransposed): v[n, j]
                for nt in range(2):
                    pv = pod.tile([128, 512], F32, tag="pod", name="pv")
                    for it in range(4):
                        nc.tensor.matmul(
                            pv[:],
                            lhsT=xt[:, it, nt * 128 : (nt + 1) * 128],
                            rhs=wq_sb[:, 8:12, it, :],
                            start=(it == 0),
                            stop=(it == 3),
                        )
                    if o["vt_on_act"]:
                        nc.scalar.copy(out=vt[:, nt, :], in_=pv[:])
                    else:
                        nc.vector.tensor_copy(out=vt[:, nt, :], in_=pv[:])

                # stage zero-padded q tiles (pure DMA, no engine cost);
                # head h = 4g+hp lives at partitions [32hp, 32hp+32),
                # dst slot h, src block g.
                qz = q0[b % o["q0_bufs"]]
                if o["q0_merged_dma"]:
                    qz_g = qz.rearrange("p (g q) n -> p g q n", q=4)
                    for hp in range(4):
                        nc.sync.dma_start(
                            out=qz_g[32 * hp : 32 * (hp + 1), :, hp, :],
                            in_=qkT[32 * hp : 32 * (hp + 1), 0:1024].rearrange(
                                "p (g n) -> p g n", n=256
                            ),
                        )
                else:
                    for h in range(H):
                        hp, g = h % 4, h // 4
                        nc.sync.dma_start(
                            out=qz[32 * hp : 32 * (hp + 1), h, :],
                            in_=qkT[32 * hp : 32 * (hp + 1), g * 256 : (g + 1) * 256],
                        )

                attn = [
                    attnpool.tile([128, 4096], BF16, tag=f"attn{mt}", name=f"attn{mt}")
                    for mt in range(2)
                ]
                return dict(xt=xt, qkT=qkT, vt=vt, qz=qz, attn=attn, b=b)

            def emit_dots_group(st, g, last=False):
                """Packed dots + exp + bias multiply for head-group g."""
                qkT, qz, attn = st["qkT"], st["qz"], st["attn"]
                for mt in range(2):
                    pd = pbig.tile([128, 1024], F32, tag="pd", name="pd")
                    for t in range(2):
                        nc.tensor.matmul(
                            pd[:, t * 512 : (t + 1) * 512],
                            lhsT=qkT[
                                :,
                                (4 + g) * 256 + mt * 128 : (4 + g) * 256
                                + (mt + 1) * 128,
                            ],
                            rhs=qz[:, 4 * g + 2 * t : 4 * g + 2 * t + 2, :],
                            start=True,
                            stop=True,
                        )
                    if o["exp_split"]:
                        # 512-col halves: exp(t0) starts as soon as the first
                        # dots matmul drains (bank-level dep), and the pd slot
                        # frees ~0.5us earlier for the next dots pair
                        for t in range(2):
                            lo = g * 1024 + t * 512
                            nc.scalar.activation(
                                out=attn[mt][:, lo : lo + 512],
                                in_=pd[:, t * 512 : (t + 1) * 512],
                                func=EXP,
                            )
                            eng = (
                                nc.vector
                                if last
                                or (4 * g + 2 * mt + t) % 16 < 2 * o["bias_dve_mod"]
                                else nc.gpsimd
                            )
                            eng.tensor_mul(
                                attn[mt][:, lo : lo + 512],
                                attn[mt][:, lo : lo + 512],
                                eb_sb[:, mt, lo : lo + 512],
                            )
                    else:
                        nc.scalar.activation(
                            out=attn[mt][:, g * 1024 : (g + 1) * 1024],
                            in_=pd[:],
                            func=EXP,
                        )
                        eng = (
                            nc.vector
                            if last or (2 * g + mt) % 8 < o["bias_dve_mod"]
                            else nc.gpsimd
                        )
                        eng.tensor_mul(
                            attn[mt][:, g * 1024 : (g + 1) * 1024],
                            attn[mt][:, g * 1024 : (g + 1) * 1024],
                            eb_sb[:, mt, g * 1024 : (g + 1) * 1024],
                        )

            def emit_attnv_group(st, g, outT):
                """attn@v + denominators + normalize for head-group g."""
                vt, attn = st["vt"], st["attn"]
                od = pod.tile([128, 512], F32, tag="pod", name="od")
                for mt in range(2):
                    bundle = []
                    for hp in range(4):
                        h = 4 * g + hp
                        bi = nc.tensor.matmul(
                            od[32 * hp : 32 * (hp + 1), 0:256],
                            lhsT=vt[:, mt, 32 * h : 32 * h + 32],
                            rhs=attn[mt][:, h * 256 : (h + 1) * 256],
                            start=(mt == 0),
                            stop=(mt == 1),
                            tile_position=(0, 32 * hp),
                            skip_group_check=True,
                        )
                        bundle.append(bi.ins.name)
                bundle = []
                for mt in range(2):
                    for hp in range(4):
                        h = 4 * g + hp
                        bi = nc.tensor.matmul(
                            od[32 * hp : 32 * (hp + 1), 256:512],
                            lhsT=ones32[:],
                            rhs=attn[mt][:, h * 256 : (h + 1) * 256],
                            start=(mt == 0),
                            stop=(mt == 1),
                            tile_position=(0, 32 * hp),
                            skip_group_check=True,
                        )
                        bundle.append(bi.ins.name)
                bundles.append(bundle)
                if o["stt_div"]:
                    nc.vector.tensor_tensor(
                        outT[:, g * 256 : (g + 1) * 256],
                        od[:, 0:256],
                        od[:, 256:512],
                        mybir.AluOpType.divide,
                    )
                else:
                    r = small.tile([128, 256], F32, tag="r")
                    nc.vector.reciprocal_approx_fast(out=r[:], in_=od[:, 256:512])
                    nc.vector.tensor_mul(
                        outT[:, g * 256 : (g + 1) * 256], od[:, 0:256], r[:]
                    )

            def emit_outproj(st, outT):
                b = st["b"]
                for nt in range(2):
                    py = pod.tile([128, 512], F32, tag="pod", name="py")
                    for ot in range(4):
                        nc.tensor.matmul(
                            py[:],
                            lhsT=outT[
                                :, ot * 256 + nt * 128 : ot * 256 + nt * 128 + 128
                            ],
                            rhs=w2_sb[:, ot, :],
                            start=(ot == 0),
                            stop=(ot == 3) and nt == 0,
                        )
                    if nt == 1:
                        nc.tensor.matmul(
                            py[:], lhsT=ones1[:], rhs=bo_sb[:], start=False,
                            stop=True,
                        )
                    ysb = small.tile([128, 512], F32, tag="ysb", name="ysb")
                    if nt == 0:
                        nc.vector.tensor_add(ysb[:], py[:], bout_bc[:])
                    else:
                        # DVE, not ACT: keep the strict-FIFO Scalar queue
                        # free for the exps that gate dots PSUM slots
                        nc.vector.tensor_copy(out=ysb[:], in_=py[:])
                    nc.sync.dma_start(out=y[b, nt], in_=ysb[:])

            def emit_back(st):
                outT = small.tile([128, 1024], BF16, tag="outT")
                for g in range(4):
                    emit_attnv_group(st, g, outT)
                emit_outproj(st, outT)

            if o["depth"] == 2 and o["interleave"]:
                # depth-2 pipeline: front(b) || dots(b-1) || attn@v(b-2)
                sts = []
                for rep in range(repeats):
                    for b in range(bl):
                        st = emit_front(rep, b)
                        outT = small.tile([128, 1024], BF16, tag="outT")
                        for g in range(4):
                            if sts:
                                emit_dots_group(sts[-1], g)
                            if len(sts) >= 2:
                                emit_attnv_group(sts[-2], g, outT)
                        if len(sts) >= 2:
                            emit_outproj(sts[-2], outT)
                        sts = sts[-1:] + [st]
                # drain: dots(last) + attn@v(second-last), then back(last)
                outT = small.tile([128, 1024], BF16, tag="outT")
                for g in range(4):
                    emit_dots_group(sts[-1], g, last=True)
                    if len(sts) >= 2:
                        emit_attnv_group(sts[-2], g, outT)
                if len(sts) >= 2:
                    emit_outproj(sts[-2], outT)
                emit_back(sts[-1])
            else:
                prev = None
                for rep in range(repeats):
                    for b in range(bl):
                        st = emit_front(rep, b)
                        if o["interleave"]:
                            outT = small.tile([128, 1024], BF16, tag="outT")
                            for g in range(4):
                                emit_dots_group(st, g)
                                if prev is not None:
                                    emit_attnv_group(prev, g, outT)
                            if prev is not None:
                                emit_outproj(prev, outT)
                        else:
                            for g in range(4):
                                emit_dots_group(st, g)
                            if prev is not None:
                                emit_back(prev)
                        prev = st
                if o["depth"] == 2 and not o["interleave"]:
                    pass
                emit_back(prev)

    if o["hoist"]:
        _hoist_ldw_bundles(nc, bundles)
    nc.compile()
    return nc


def _get_nc(bl: int, repeats: int = 1, opts: dict | None = None):
    key = (bl, repeats, tuple(sorted((opts or {}).items())))
    if key not in _CACHE:
        _CACHE[key] = _build(bl, repeats, opts)
    return _CACHE[key]


def _prep_inputs(x, w_qkv, rel_bias_table, w_out, b_out):
    """Host-side layout prep: transpose/tile/bf16-cast, bias-table gather."""
    x = np.asarray(x, np.float32)
    w_qkv = np.asarray(w_qkv, np.float32).copy()
    rel_bias_table = np.asarray(rel_bias_table, np.float32)
    w_out = np.asarray(w_out, np.float32)
    b_out = np.asarray(b_out, np.float32)

    # fold the attention scale into the q columns of w_qkv
    w_qkv[:, :OUP] *= SCALE

    # xT_dev[b, p, it, n] = x[b, n, it*128+p]
    xT = np.ascontiguousarray(
        x.transpose(0, 2, 1).reshape(B, 4, 128, N).transpose(0, 2, 1, 3)
    ).astype(NPBF16)
    # wqkv_dev[p, jt, it, jj] = w_qkv[it*128+p, jt*128+jj]
    wqkv_dev = np.ascontiguousarray(
        w_qkv.reshape(4, 128, 12, 128).transpose(1, 2, 0, 3)
    ).astype(NPBF16)
    # w2t_dev[p, ot, q] = w_out.T[ot*128+p, q] = w_out[q, ot*128+p]
    w2t_dev = np.ascontiguousarray(
        w_out.T.reshape(4, 128, OUP).transpose(1, 0, 2)
    ).astype(NPBF16)
    # bias[n, m, h]; ebT_dev[p, mt, h*256+n] = exp(bias[n, mt*128+p, h])
    rel_idx = _relative_index(16, 16)
    bias = rel_bias_table[rel_idx].reshape(N, N, H)  # [n, m, h]
    ebT = np.exp(bias.transpose(2, 1, 0))  # [h, m, n]
    ebT_dev = np.ascontiguousarray(
        ebT.reshape(H, 2, 128, N).transpose(2, 1, 0, 3).reshape(128, 2, H * N)
    ).astype(NPBF16)
    bout_dev = b_out.reshape(1, OUP).astype(np.float32)
    return xT, wqkv_dev, w2t_dev, ebT_dev, bout_dev


def kernel(x, w_qkv, rel_bias_table, w_out, b_out, ih, iw):
    assert int(ih) == 16 and int(iw) == 16
    xT, wqkv_dev, w2t_dev, ebT_dev, bout_dev = _prep_inputs(
        x, w_qkv, rel_bias_table, w_out, b_out
    )

    nc = _get_nc(BL)
    in_maps = []
    for c in range(NCORES):
        in_maps.append(
            {
                "xT": np.ascontiguousarray(xT[c * BL : (c + 1) * BL]),
                "wqkv": wqkv_dev,
                "w2t": w2t_dev,
                "ebT": ebT_dev,
                "bout": bout_dev,
                "boutb": bout_dev.astype(NPBF16),
            }
        )

    trace = bool(os.environ.get("BASS_TRACE_KERNEL"))
    if trace:
        try:
            from antenv.axon_hooks import get_axon_ntff_profile_hook  # noqa: F401
        except ImportError:
            trace = False
    res = run_bass_kernel_spmd(nc, in_maps, core_ids=list(range(NCORES)), trace=trace)
    kernel.last_result = res
    if res.exec_time_ns is not None:
        print(f"HW exec time: {res.exec_time_ns} ns")

    y = np.concatenate(
        [r["y"].reshape(BL, N, OUP) for r in res.results], axis=0
    ).astype(np.float32)
    return y


kernel.last_result = None


# revision 42
# speedup vs baseline: 1.0197x; 1.0197x over previous
"""Trainium2 Bass kernel for nn_Attention (dense transformer block).

Strategy: data-parallel over batch across 8 NeuronCores (8 batches/core).
Per core, per batch (N=256 tokens, 16 heads, dim_head=32):
  - qkv projection: q,k computed TRANSPOSED (qkT [j, n], weight-stationary),
    v computed untransposed (x^T-stationary) -> v [n, j] so the attn@v matmul
    needs no on-chip transposes at all.
  - dots^T[m, n] = k_h^T.T-stationary @ q_h^T streaming, K=32, 4 heads packed
    into the 4 PE row-groups via a zero-padded q staging buffer (row-tiled
    matmuls at nonzero tile_position fault this HW/compiler combo).
  - softmax without max-subtraction (|dots| <~ 1.5 by construction) and with
    normalization deferred: attn_unnorm = exp(dots^T) * exp(bias^T) (exp on
    ACT reading PSUM directly, bias multiply split DVE/GPSIMD in bf16).
  - attn@v: out_h^T[d, n] = v_h[m, d]-stationary @ attn^T streaming, 4 heads
    packed into PE col-groups (emitted back-to-back so the col-group matmuls
    execute concurrently in disjoint column groups of the PE array); a
    parallel ones[m,32]-stationary matmul computes the softmax denominators
    as a 32-row broadcast.
  - out projection with b_out folded in as a K=1 matmul row; PSUM -> DRAM DMA.
  - depth-2 software pipeline: front(b) [qkv proj + q0 staging] runs with
    dots(b-1) and attn@v/outproj(b-2) interleaved per head-group, hiding the
    q0-staging DMA latency and the exp/bias-mul latency under PE work.
  - j-tile-progressive wqkv loads (j-tile-major DRAM layout) let the first
    projection matmul start ~10us earlier at kernel startup.
All matmuls in bf16 (fp32 PSUM accumulation); rel-err vs fp32 reference ~1e-3.
"""

import os
import sys

import numpy as np

if "/opt/trn_rl_repo" not in sys.path:
    sys.path.insert(0, "/opt/trn_rl_repo")

import ml_dtypes  # noqa: E402

from concourse import bacc, mybir  # noqa: E402
from concourse.tile import TileContext  # noqa: E402
from concourse.bass_utils import run_bass_kernel_spmd  # noqa: E402

BF16 = mybir.dt.bfloat16
F32 = mybir.dt.float32
NPBF16 = ml_dtypes.bfloat16

B, N, INP, OUP, H, D = 64, 256, 512, 512, 16, 32
NCORES = 8
BL = B // NCORES  # batches per core
SCALE = D ** -0.5

_CACHE = {}


def _relative_index(ih: int, iw: int) -> np.ndarray:
    yy, xx = np.meshgrid(np.arange(ih), np.arange(iw), indexing="ij")
    coords = np.stack([yy.ravel(), xx.ravel()])
    rel = coords[:, :, None] - coords[:, None, :]
    rel[0] += ih - 1
    rel[1] += iw - 1
    rel[0] *= 2 * iw - 1
    return rel.sum(0).ravel()


DEFAULT_OPTS = {
    "bias_dve_mod": 3,      # (2g+mt) % 8 < this -> DVE, else GPSIMD
    "hoist": False,         # LDW hoisting: measured neutral, keep off
    "interleave": True,     # software-pipeline attn@v(b-1) under dots(b)
    "q0_merged_dma": True,  # 4 merged q0 DMAs vs 16 per-head
    "depth": 2,             # software-pipeline depth (1 or 2)
    "stt_div": False,       # DVE divide: rejected by walrus codegen on TRN2
    "vt_on_act": False,     # ACT-queue FIFO delays vt behind exps: keep on DVE
    "pd_bufs": 2,
    "unified_pd": False,
    "pod_bufs": 2,
    "xpool_bufs": 3,
    "qkv_bufs": 3,
    "attn_bufs": 2,
    "evac_chunks": 2,
    "exp_split": False,     # 512-col exp/mul halves: measured slower
    "small_bufs": 4,
    "q0_bufs": 2,
}


def _hoist_ldw_bundles(nc, bundles):
    """Reorder each bundle's [LDW MM LDW MM ...] pairs to [LDW... MM...].

    Only the PE-stream relative order changes; LDWEIGHTS carry no semaphore
    updates, so every cross-engine wait threshold keeps its meaning.  The
    instruction objects are written back into the same block slots, leaving
    other engines' instructions untouched.
    """
    fn = nc.m.functions[0]
    n_done = 0
    for blk in fn.blocks:
        ilist = blk.instructions
        name2idx = {inst.name: k for k, inst in enumerate(ilist)}
        for bundle in bundles:
            idxs = [name2idx.get(n) for n in bundle]
            if any(i is None for i in idxs):
                continue
            slots = []
            ok = True
            for i in idxs:
                if i == 0 or not isinstance(ilist[i - 1], mybir.InstLdweights):
                    ok = False
                    break
                slots.extend([i - 1, i])
            if not ok:
                continue
            # a foreign PE instruction inside the span would have its
            # LDWEIGHTS clobber the hoisted col strips -> only hoist
            # bundles whose pair slots are contiguous on the PE stream
            slot_set = set(slots)
            for k in range(min(slots), max(slots) + 1):
                if k not in slot_set and ilist[k].engine == mybir.EngineType.PE:
                    ok = False
                    break
            if not ok:
                continue
            slots_sorted = sorted(slots)
            ldws = [ilist[i - 1] for i in idxs]
            mms = [ilist[i] for i in idxs]
            new_order = ldws + mms
            for slot, inst in zip(slots_sorted, new_order):
                ilist[slot] = inst
            n_done += 1
    if os.environ.get("BASS_DEBUG_HOIST"):
        print(f"hoisted {n_done}/{len(bundles)} LDW bundles")
    return n_done


def _build(bl: int, repeats: int = 1, opts: dict | None = None):
    o = dict(DEFAULT_OPTS)
    if opts:
        o.update(opts)
    nc = bacc.Bacc(None, target_bir_lowering=False)

    xT = nc.declare_dram_parameter("xT", [bl, 128, 4, 256], BF16, isOutput=False)
    # wqkv[p, jt, it, jj] = w_qkv[it*128+p, jt*128+jj] -- j-tile-major so the
    # per-j-tile startup loads are contiguous
    wqkv = nc.declare_dram_parameter("wqkv", [128, 12, 4, 128], BF16, isOutput=False)
    w2t = nc.declare_dram_parameter("w2t", [128, 4, 512], BF16, isOutput=False)
    ebT = nc.declare_dram_parameter("ebT", [128, 2, 4096], BF16, isOutput=False)
    bout = nc.declare_dram_parameter("bout", [1, 512], F32, isOutput=False)
    boutb = nc.declare_dram_parameter("boutb", [1, 512], BF16, isOutput=False)
    y = nc.declare_dram_parameter("y", [bl, 2, 128, 512], F32, isOutput=True)

    EXP = mybir.ActivationFunctionType.Exp
    bundles = []  # lists of InstMatmult names whose LDWs get hoisted

    with TileContext(nc) as tc:
        with (
            tc.tile_pool(name="consts", bufs=1) as consts,
            tc.tile_pool(name="xpool", bufs=o["xpool_bufs"]) as xpool,
            tc.tile_pool(name="qkvpool", bufs=o["qkv_bufs"]) as qkvpool,
            tc.tile_pool(name="attnpool", bufs=o["attn_bufs"]) as attnpool,
            tc.tile_pool(name="small", bufs=o["small_bufs"]) as small,
            tc.tile_pool(name="pbig", bufs=o["pd_bufs"], space="PSUM") as pbig,
            tc.tile_pool(name="pod", bufs=o["pod_bufs"], space="PSUM") as pod,
        ):
            # constant loads spread across engine DMA queues so the first
            # batch's x tile (sync queue) isn't stuck behind them
            # j-tile-progressive weight loads: the first projection matmul
            # group needs only x and wq j-tile 0, so it can start as soon as
            # those land instead of waiting for the full 1.5 MB of wqkv.
            xt_pre = xpool.tile([128, 4, 256], BF16, tag="xt", name="xt")
            nc.sync.dma_start(xt_pre[:, 0:2, :], xT[0, :, 0:2, :])
            nc.scalar.dma_start(xt_pre[:, 2:4, :], xT[0, :, 2:4, :])
            wq_sb = consts.tile([128, 12, 4, 128], BF16)
            # jt0 on the SWDGE queue: it starts flowing earliest and the
            # bias table behind it isn't needed until the first bias-mul
            nc.gpsimd.dma_start(wq_sb[:, 0], wqkv[:, 0])
            for jt in range(1, 4):
                eng = nc.scalar if jt % 2 == 0 else nc.sync
                eng.dma_start(wq_sb[:, jt], wqkv[:, jt])
            nc.sync.dma_start(wq_sb[:, 4:8], wqkv[:, 4:8])
            nc.scalar.dma_start(wq_sb[:, 8:12], wqkv[:, 8:12])
            eb_sb = consts.tile([128, 2, 4096], BF16)
            nc.gpsimd.dma_start(eb_sb[:], ebT[:])
            w2_sb = consts.tile([128, 4, 512], BF16)
            nc.scalar.dma_start(w2_sb[:], w2t[:])
            bout_bc = consts.tile([128, 512], F32)
            nc.scalar.dma_start(bout_bc[:], bout[:].to_broadcast((128, 512)))
            bo_sb = consts.tile([1, 512], BF16)
            nc.scalar.dma_start(bo_sb[:], boutb[:])
            ones32 = consts.tile([128, 32], BF16)
            nc.vector.memset(ones32[:], 1.0)
            ones1 = consts.tile([1, 128], BF16)
            nc.vector.memset(ones1[:], 1.0)
            # zero-padded q staging: q0[p, h, n] nonzero only for
            # p in [32*(h%4), 32*(h%4)+32); the zero rows are written once
            # and never touched again (per-batch DMAs overwrite only the
            # nonzero rows), so the cross-head terms of the full-K dots
            # matmuls vanish. Two buffers, alternated by batch parity.
            q0 = []
            for i in range(o["q0_bufs"]):
                t = consts.tile([128, 16, 256], BF16, name=f"q0_{i}")
                nc.vector.memset(t[:], 0.0)
                q0.append(t)

            def emit_front(rep, b):
                """qkv projection + dots + exp + bias-mul for batch b."""
                if rep == 0 and b == 0:
                    xt = xt_pre
                else:
                    xt = xpool.tile([128, 4, 256], BF16, tag="xt", name="xt")
                    nc.sync.dma_start(xt[:], xT[b])

                qkT = qkvpool.tile([128, 2048], BF16, tag="qkT")
                vt = qkvpool.tile([128, 2, 512], BF16, tag="vt")

                # q,k projection (transposed): out[j, n] over j-tiles 0..7
                for half in range(2):
                    pqk = pbig.tile(
                        [128, 1024], F32, name="pqk",
                        **({"tag": "pd"} if o["unified_pd"] else {"tag": "pqk", "bufs": 1}),
                    )
                    for jq in range(4):
                        jt = half * 4 + jq
                        for it in range(4):
                            nc.tensor.matmul(
                                pqk[:, jq * 256 : (jq + 1) * 256],
                                lhsT=wq_sb[:, jt, it, :],
                                rhs=xt[:, it, :],
                                start=(it == 0),
                                stop=(it == 3),
                            )
                    if o["evac_chunks"] == 1:
                        nc.vector.tensor_copy(
                            out=qkT[:, half * 1024 : (half + 1) * 1024], in_=pqk[:]
                        )
                    else:
                        ch = 1024 // o["evac_chunks"]
                        for ci in range(o["evac_chunks"]):
                            nc.vector.tensor_copy(
                                out=qkT[
                                    :,
                                    half * 1024 + ci * ch : half * 1024
                                    + (ci + 1) * ch,
                                ],
                                in_=pqk[:, ci * ch : (ci + 1) * ch],
                            )

                # v projection (untransposed): v[n, j]
                for nt in range(2):
                    pv = pod.tile([128, 512], F32, tag="pod", name="pv")
                    for it in range(4):
                        nc.tensor.matmul(
                            pv[:],
                            lhsT=xt[:, it, nt * 128 : (nt + 1) * 128],
                            rhs=wq_sb[:, 8:12, it, :],
                            start=(it == 0),
                            stop=(it == 3),
                        )
                    if o["vt_on_act"]:
                        nc.scalar.copy(out=vt[:, nt, :], in_=pv[:])
                    else:
                        nc.vector.tensor_copy(out=vt[:, nt, :], in_=pv[:])

                # stage zero-padded q tiles (pure DMA, no engine cost);
                # head h = 4g+hp lives at partitions [32hp, 32hp+32),
                # dst slot h, src block g.
                qz = q0[b % o["q0_bufs"]]
                if o["q0_merged_dma"]:
                    qz_g = qz.rearrange("p (g q) n -> p g q n", q=4)
                    for hp in range(4):
                        nc.sync.dma_start(
                            out=qz_g[32 * hp : 32 * (hp + 1), :, hp, :],
                            in_=qkT[32 * hp : 32 * (hp + 1), 0:1024].rearrange(
                                "p (g n) -> p g n", n=256
                            ),
                        )
                else:
                    for h in range(H):
                        hp, g = h % 4, h // 4
                        nc.sync.dma_start(
                            out=qz[32 * hp : 32 * (hp + 1), h, :],
                            in_=qkT[32 * hp : 32 * (hp + 1), g * 256 : (g + 1) * 256],
                        )

                attn = [
                    attnpool.tile([128, 4096], BF16, tag=f"attn{mt}", name=f"attn{mt}")
                    for mt in range(2)
                ]
                return dict(xt=xt, qkT=qkT, vt=vt, qz=qz, attn=attn, b=b)

            def emit_dots_group(st, g, last=False):
                """Packed dots + exp + bias multiply for head-group g."""
                qkT, qz, attn = st["qkT"], st["qz"], st["attn"]
                for mt in range(2):
                    pd = pbig.tile([128, 1024], F32, tag="pd", name="pd")
                    for t in range(2):
                        nc.tensor.matmul(
                            pd[:, t * 512 : (t + 1) * 512],
                            lhsT=qkT[
                                :,
                                (4 + g) * 256 + mt * 128 : (4 + g) * 256
                                + (mt + 1) * 128,
                            ],
                            rhs=qz[:, 4 * g + 2 * t : 4 * g + 2 * t + 2, :],
                            start=True,
                            stop=True,
                        )
                    if o["exp_split"]:
                        # 512-col halves: exp(t0) starts as soon as the first
                        # dots matmul drains (bank-level dep), and the pd slot
                        # frees ~0.5us earlier for the next dots pair
                        for t in range(2):
                            lo = g * 1024 + t * 512
                            nc.scalar.activation(
                                out=attn[mt][:, lo : lo + 512],
                                in_=pd[:, t * 512 : (t + 1) * 512],
                                func=EXP,
                            )
                            eng = (
                                nc.vector
                                if last
                                or (4 * g + 2 * mt + t) % 16 < 2 * o["bias_dve_mod"]
                                else nc.gpsimd
                            )
                            eng.tensor_mul(
                                attn[mt][:, lo : lo + 512],
                                attn[mt][:, lo : lo + 512],
                                eb_sb[:, mt, lo : lo + 512],
                            )
                    else:
                        nc.scalar.activation(
                            out=attn[mt][:, g * 1024 : (g + 1) * 1024],
                            in_=pd[:],
                            func=EXP,
                        )
                        eng = (
                            nc.vector
                            if last or (2 * g + mt) % 8 < o["bias_dve_mod"]
                            else nc.gpsimd
                        )
                        eng.tensor_mul(
                            attn[mt][:, g * 1024 : (g + 1) * 1024],
                            attn[mt][:, g * 1024 : (g + 1) * 1024],
                            eb_sb[:, mt, g * 1024 : (g + 1) * 1024],
                        )

            def emit_attnv_group(st, g, outT):
                """attn@v + denominators + normalize for head-group g."""
                vt, attn = st["vt"], st["attn"]
                od = pod.tile([128, 512], F32, tag="pod", name="od")
                for mt in range(2):
                    bundle = []
                    for hp in range(4):
                        h = 4 * g + hp
                        bi = nc.tensor.matmul(
                            od[32 * hp : 32 * (hp + 1), 0:256],
                            lhsT=vt[:, mt, 32 * h : 32 * h + 32],
                            rhs=attn[mt][:, h * 256 : (h + 1) * 256],
                            start=(mt == 0),
                            stop=(mt == 1),
                            tile_position=(0, 32 * hp),
                            skip_group_check=True,
                        )
                        bundle.append(bi.ins.name)
                bundle = []
                for mt in range(2):
                    for hp in range(4):
                        h = 4 * g + hp
                        bi = nc.tensor.matmul(
                            od[32 * hp : 32 * (hp + 1), 256:512],
                            lhsT=ones32[:],
                            rhs=attn[mt][:, h * 256 : (h + 1) * 256],
                            start=(mt == 0),
                            stop=(mt == 1),
                            tile_position=(0, 32 * hp),
                            skip_group_check=True,
                        )
                        bundle.append(bi.ins.name)
                bundles.append(bundle)
                if o["stt_div"]:
                    nc.vector.tensor_tensor(
                        outT[:, g * 256 : (g + 1) * 256],
                        od[:, 0:256],
                        od[:, 256:512],
                        mybir.AluOpType.divide,
                    )
                else:
                    r = small.tile([128, 256], F32, tag="r")
                    nc.vector.reciprocal_approx_fast(out=r[:], in_=od[:, 256:512])
                    nc.vector.tensor_mul(
                        outT[:, g * 256 : (g + 1) * 256], od[:, 0:256], r[:]
                    )

            def emit_outproj(st, outT):
                b = st["b"]
                for nt in range(2):
                    py = pod.tile([128, 512], F32, tag="pod", name="py")
                    for ot in range(4):
                        nc.tensor.matmul(
                            py[:],
                            lhsT=outT[
                                :, ot * 256 + nt * 128 : ot * 256 + nt * 128 + 128
                            ],
                            rhs=w2_sb[:, ot, :],
                            start=(ot == 0),
                            stop=(ot == 3),
                        )
                    ysb = small.tile([128, 512], F32, tag="ysb", name="ysb")
                    # bias add rides the DVE evacuation (same cost as a copy);
                    # DVE, not ACT, keeps the strict-FIFO Scalar queue free
                    # for the exps that gate dots PSUM slots
                    nc.vector.tensor_add(ysb[:], py[:], bout_bc[:])
                    nc.sync.dma_start(out=y[b, nt], in_=ysb[:])

            def emit_back(st):
                outT = small.tile([128, 1024], BF16, tag="outT")
                for g in range(4):
                    emit_attnv_group(st, g, outT)
                emit_outproj(st, outT)

            if o["depth"] == 2 and o["interleave"]:
                # depth-2 pipeline: front(b) || dots(b-1) || attn@v(b-2)
                sts = []
                for rep in range(repeats):
                    for b in range(bl):
                        st = emit_front(rep, b)
                        outT = small.tile([128, 1024], BF16, tag="outT")
                        for g in range(4):
                            if sts:
                                emit_dots_group(sts[-1], g)
                            if len(sts) >= 2:
                                emit_attnv_group(sts[-2], g, outT)
                        if len(sts) >= 2:
                            emit_outproj(sts[-2], outT)
                        sts = sts[-1:] + [st]
                # drain: dots(last) + attn@v(second-last), then back(last)
                outT = small.tile([128, 1024], BF16, tag="outT")
                for g in range(4):
                    emit_dots_group(sts[-1], g, last=True)
                    if len(sts) >= 2:
                        emit_attnv_group(sts[-2], g, outT)
                if len(sts) >= 2:
                    emit_outproj(sts[-2], outT)
                emit_back(sts[-1])
            else:
                prev = None
                for rep in range(repeats):
                    for b in range(bl):
                        st = emit_front(rep, b)
                        if o["interleave"]:
                            outT = small.tile([128, 1024], BF16, tag="outT")
                            for g in range(4):
                                emit_dots_group(st, g)
                                if prev is not None:
                                    emit_attnv_group(prev, g, outT)
                            if prev is not None:
                                emit_outproj(prev, outT)
                        else:
                            for g in range(4):
                                emit_dots_group(st, g)
                            if prev is not None:
                                emit_back(prev)
                        prev = st
                if o["depth"] == 2 and not o["interleave"]:
                    pass
                emit_back(prev)

    if o["hoist"]:
        _hoist_ldw_bundles(nc, bundles)
    nc.compile()
    return nc


def _get_nc(bl: int, repeats: int = 1, opts: dict | None = None):
    key = (bl, repeats, tuple(sorted((opts or {}).items())))
    if key not in _CACHE:
        _CACHE[key] = _build(bl, repeats, opts)
    return _CACHE[key]


def _prep_inputs(x, w_qkv, rel_bias_table, w_out, b_out):
    """Host-side layout prep: transpose/tile/bf16-cast, bias-table gather."""
    x = np.asarray(x, np.float32)
    w_qkv = np.asarray(w_qkv, np.float32).copy()
    rel_bias_table = np.asarray(rel_bias_table, np.float32)
    w_out = np.asarray(w_out, np.float32)
    b_out = np.asarray(b_out, np.float32)

    # fold the attention scale into the q columns of w_qkv
    w_qkv[:, :OUP] *= SCALE

    # xT_dev[b, p, it, n] = x[b, n, it*128+p]
    xT = np.ascontiguousarray(
        x.transpose(0, 2, 1).reshape(B, 4, 128, N).transpose(0, 2, 1, 3)
    ).astype(NPBF16)
    # wqkv_dev[p, jt, it, jj] = w_qkv[it*128+p, jt*128+jj]
    wqkv_dev = np.ascontiguousarray(
        w_qkv.reshape(4, 128, 12, 128).transpose(1, 2, 0, 3)
    ).astype(NPBF16)
    # w2t_dev[p, ot, q] = w_out.T[ot*128+p, q] = w_out[q, ot*128+p]
    w2t_dev = np.ascontiguousarray(
        w_out.T.reshape(4, 128, OUP).transpose(1, 0, 2)
    ).astype(NPBF16)
    # bias[n, m, h]; ebT_dev[p, mt, h*256+n] = exp(bias[n, mt*128+p, h])
    rel_idx = _relative_index(16, 16)
    bias = rel_bias_table[rel_idx].reshape(N, N, H)  # [n, m, h]
    ebT = np.exp(bias.transpose(2, 1, 0))  # [h, m, n]
    ebT_dev = np.ascontiguousarray(
        ebT.reshape(H, 2, 128, N).transpose(2, 1, 0, 3).reshape(128, 2, H * N)
    ).astype(NPBF16)
    bout_dev = b_out.reshape(1, OUP).astype(np.float32)
    return xT, wqkv_dev, w2t_dev, ebT_dev, bout_dev


def kernel(x, w_qkv, rel_bias_table, w_out, b_out, ih, iw):
    assert int(ih) == 16 and int(iw) == 16
    xT, wqkv_dev, w2t_dev, ebT_dev, bout_dev = _prep_inputs(
        x, w_qkv, rel_bias_table, w_out, b_out
    )

    nc = _get_nc(BL)
    in_maps = []
    for c in range(NCORES):
        in_maps.append(
            {
                "xT": np.ascontiguousarray(xT[c * BL : (c + 1) * BL]),
                "wqkv": wqkv_dev,
                "w2t": w2t_dev,
                "ebT": ebT_dev,
                "bout": bout_dev,
                "boutb": bout_dev.astype(NPBF16),
            }
        )

    trace = bool(os.environ.get("BASS_TRACE_KERNEL"))
    if trace:
        try:
            from antenv.axon_hooks import get_axon_ntff_profile_hook  # noqa: F401
        except ImportError:
            trace = False
    res = run_bass_kernel_spmd(nc, in_maps, core_ids=list(range(NCORES)), trace=trace)
    kernel.last_result = res
    if res.exec_time_ns is not None:
        print(f"HW exec time: {res.exec_time_ns} ns")

    y = np.concatenate(
        [r["y"].reshape(BL, N, OUP) for r in res.results], axis=0
    ).astype(np.float32)
    return y


kernel.last_result = None


# revision 43
# speedup vs baseline: 1.0225x; 1.0027x over previous
"""Trainium2 Bass kernel for nn_Attention (dense transformer block).

Strategy: data-parallel over batch across 8 NeuronCores (8 batches/core).
Per core, per batch (N=256 tokens, 16 heads, dim_head=32):
  - qkv projection: q,k computed TRANSPOSED (qkT [j, n], weight-stationary),
    v computed untransposed (x^T-stationary) -> v [n, j] so the attn@v matmul
    needs no on-chip transposes at all.
  - dots^T[m, n] = k_h^T.T-stationary @ q_h^T streaming, K=32, 4 heads packed
    into the 4 PE row-groups via a zero-padded q staging buffer (row-tiled
    matmuls at nonzero tile_position fault this HW/compiler combo).
  - softmax without max-subtraction (|dots| <~ 1.5 by construction) and with
    normalization deferred: attn_unnorm = exp(dots^T) * exp(bias^T) (exp on
    ACT reading PSUM directly, bias multiply split DVE/GPSIMD in bf16).
  - attn@v: out_h^T[d, n] = v_h[m, d]-stationary @ attn^T streaming, 4 heads
    packed into PE col-groups (emitted back-to-back so the col-group matmuls
    execute concurrently in disjoint column groups of the PE array); a
    parallel ones[m,32]-stationary matmul computes the softmax denominators
    as a 32-row broadcast.
  - out projection; b_out rides the DVE PSUM-evacuation add; PSUM -> DRAM DMA.
  - depth-2 software pipeline: front(b) [qkv proj + q0 staging] runs with
    dots(b-1) and attn@v/outproj(b-2) interleaved per head-group, hiding the
    q0-staging DMA latency and the exp/bias-mul latency under PE work.
  - j-tile-progressive wqkv loads (j-tile-major DRAM layout) let the first
    projection matmul start ~10us earlier at kernel startup.
All matmuls in bf16 (fp32 PSUM accumulation); rel-err vs fp32 reference ~1e-3.
"""

import os
import sys

import numpy as np

if "/opt/trn_rl_repo" not in sys.path:
    sys.path.insert(0, "/opt/trn_rl_repo")

import ml_dtypes  # noqa: E402

from concourse import bacc, mybir  # noqa: E402
from concourse.tile import TileContext  # noqa: E402
from concourse.bass_utils import run_bass_kernel_spmd  # noqa: E402

BF16 = mybir.dt.bfloat16
F32 = mybir.dt.float32
NPBF16 = ml_dtypes.bfloat16

B, N, INP, OUP, H, D = 64, 256, 512, 512, 16, 32
NCORES = 8
BL = B // NCORES  # batches per core
SCALE = D ** -0.5

_CACHE = {}


def _relative_index(ih: int, iw: int) -> np.ndarray:
    yy, xx = np.meshgrid(np.arange(ih), np.arange(iw), indexing="ij")
    coords = np.stack([yy.ravel(), xx.ravel()])
    rel = coords[:, :, None] - coords[:, None, :]
    rel[0] += ih - 1
    rel[1] += iw - 1
    rel[0] *= 2 * iw - 1
    return rel.sum(0).ravel()


DEFAULT_OPTS = {
    "bias_dve_mod": 3,      # (2g+mt) % 8 < this -> DVE, else GPSIMD
    "hoist": False,         # LDW hoisting: measured neutral, keep off
    "interleave": True,     # software-pipeline attn@v(b-1) under dots(b)
    "q0_merged_dma": True,  # 4 merged q0 DMAs vs 16 per-head
    "depth": 2,             # software-pipeline depth (1 or 2)
    "stt_div": False,       # DVE divide: rejected by walrus codegen on TRN2
    "vt_on_act": False,     # ACT-queue FIFO delays vt behind exps: keep on DVE
    "pd_bufs": 2,
    "unified_pd": False,
    "pod_bufs": 2,
    "xpool_bufs": 3,
    "qkv_bufs": 3,
    "attn_bufs": 2,
    "evac_chunks": 2,
    "exp_split": False,     # 512-col exp/mul halves: measured slower
    "small_bufs": 4,
    "q0_bufs": 2,
}


def _hoist_ldw_bundles(nc, bundles):
    """Reorder each bundle's [LDW MM LDW MM ...] pairs to [LDW... MM...].

    Only the PE-stream relative order changes; LDWEIGHTS carry no semaphore
    updates, so every cross-engine wait threshold keeps its meaning.  The
    instruction objects are written back into the same block slots, leaving
    other engines' instructions untouched.
    """
    fn = nc.m.functions[0]
    n_done = 0
    for blk in fn.blocks:
        ilist = blk.instructions
        name2idx = {inst.name: k for k, inst in enumerate(ilist)}
        for bundle in bundles:
            idxs = [name2idx.get(n) for n in bundle]
            if any(i is None for i in idxs):
                continue
            slots = []
            ok = True
            for i in idxs:
                if i == 0 or not isinstance(ilist[i - 1], mybir.InstLdweights):
                    ok = False
                    break
                slots.extend([i - 1, i])
            if not ok:
                continue
            # a foreign PE instruction inside the span would have its
            # LDWEIGHTS clobber the hoisted col strips -> only hoist
            # bundles whose pair slots are contiguous on the PE stream
            slot_set = set(slots)
            for k in range(min(slots), max(slots) + 1):
                if k not in slot_set and ilist[k].engine == mybir.EngineType.PE:
                    ok = False
                    break
            if not ok:
                continue
            slots_sorted = sorted(slots)
            ldws = [ilist[i - 1] for i in idxs]
            mms = [ilist[i] for i in idxs]
            new_order = ldws + mms
            for slot, inst in zip(slots_sorted, new_order):
                ilist[slot] = inst
            n_done += 1
    if os.environ.get("BASS_DEBUG_HOIST"):
        print(f"hoisted {n_done}/{len(bundles)} LDW bundles")
    return n_done


def _build(bl: int, repeats: int = 1, opts: dict | None = None):
    o = dict(DEFAULT_OPTS)
    if opts:
        o.update(opts)
    nc = bacc.Bacc(None, target_bir_lowering=False)

    xT = nc.declare_dram_parameter("xT", [bl, 128, 4, 256], BF16, isOutput=False)
    # wqkv[p, jt, it, jj] = w_qkv[it*128+p, jt*128+jj] -- j-tile-major so the
    # per-j-tile startup loads are contiguous
    wqkv = nc.declare_dram_parameter("wqkv", [128, 12, 4, 128], BF16, isOutput=False)
    w2t = nc.declare_dram_parameter("w2t", [128, 4, 512], BF16, isOutput=False)
    ebT = nc.declare_dram_parameter("ebT", [128, 2, 4096], BF16, isOutput=False)
    bout = nc.declare_dram_parameter("bout", [1, 512], F32, isOutput=False)
    boutb = nc.declare_dram_parameter("boutb", [1, 512], BF16, isOutput=False)
    y = nc.declare_dram_parameter("y", [bl, 2, 128, 512], F32, isOutput=True)

    EXP = mybir.ActivationFunctionType.Exp
    bundles = []  # lists of InstMatmult names whose LDWs get hoisted

    with TileContext(nc) as tc:
        with (
            tc.tile_pool(name="consts", bufs=1) as consts,
            tc.tile_pool(name="xpool", bufs=o["xpool_bufs"]) as xpool,
            tc.tile_pool(name="qkvpool", bufs=o["qkv_bufs"]) as qkvpool,
            tc.tile_pool(name="attnpool", bufs=o["attn_bufs"]) as attnpool,
            tc.tile_pool(name="small", bufs=o["small_bufs"]) as small,
            tc.tile_pool(name="pbig", bufs=o["pd_bufs"], space="PSUM") as pbig,
            tc.tile_pool(name="pod", bufs=o["pod_bufs"], space="PSUM") as pod,
        ):
            # constant loads spread across engine DMA queues so the first
            # batch's x tile (sync queue) isn't stuck behind them
            # j-tile-progressive weight loads: the first projection matmul
            # group needs only x and wq j-tile 0, so it can start as soon as
            # those land instead of waiting for the full 1.5 MB of wqkv.
            xt_pre = xpool.tile([128, 4, 256], BF16, tag="xt", name="xt")
            nc.sync.dma_start(xt_pre[:, 0:2, :], xT[0, :, 0:2, :])
            nc.scalar.dma_start(xt_pre[:, 2:4, :], xT[0, :, 2:4, :])
            wq_sb = consts.tile([128, 12, 4, 128], BF16)
            # jt0 on the SWDGE queue: it starts flowing earliest and the
            # bias table behind it isn't needed until the first bias-mul
            nc.gpsimd.dma_start(wq_sb[:, 0], wqkv[:, 0])
            for jt in range(1, 4):
                eng = nc.scalar if jt % 2 == 0 else nc.sync
                eng.dma_start(wq_sb[:, jt], wqkv[:, jt])
            nc.sync.dma_start(wq_sb[:, 4:8], wqkv[:, 4:8])
            nc.scalar.dma_start(wq_sb[:, 8:12], wqkv[:, 8:12])
            eb_sb = consts.tile([128, 2, 4096], BF16)
            nc.gpsimd.dma_start(eb_sb[:], ebT[:])
            w2_sb = consts.tile([128, 4, 512], BF16)
            nc.scalar.dma_start(w2_sb[:], w2t[:])
            bout_bc = consts.tile([128, 512], F32)
            nc.scalar.dma_start(bout_bc[:], bout[:].to_broadcast((128, 512)))
            bo_sb = consts.tile([1, 512], BF16)
            nc.scalar.dma_start(bo_sb[:], boutb[:])
            ones32 = consts.tile([128, 32], BF16)
            nc.vector.memset(ones32[:], 1.0)
            ones1 = consts.tile([1, 128], BF16)
            nc.vector.memset(ones1[:], 1.0)
            # zero-padded q staging: q0[p, h, n] nonzero only for
            # p in [32*(h%4), 32*(h%4)+32); the zero rows are written once
            # and never touched again (per-batch DMAs overwrite only the
            # nonzero rows), so the cross-head terms of the full-K dots
            # matmuls vanish. Two buffers, alternated by batch parity.
            q0 = []
            for i in range(o["q0_bufs"]):
                t = consts.tile([128, 16, 256], BF16, name=f"q0_{i}")
                nc.vector.memset(t[:], 0.0)
                q0.append(t)

            def emit_front(rep, b):
                """qkv projection + dots + exp + bias-mul for batch b."""
                if rep == 0 and b == 0:
                    xt = xt_pre
                else:
                    xt = xpool.tile([128, 4, 256], BF16, tag="xt", name="xt")
                    nc.sync.dma_start(xt[:], xT[b])

                qkT = qkvpool.tile([128, 2048], BF16, tag="qkT")
                vt = qkvpool.tile([128, 2, 512], BF16, tag="vt")

                # q,k projection (transposed): out[j, n] over j-tiles 0..7
                for half in range(2):
                    pqk = pbig.tile(
                        [128, 1024], F32, name="pqk",
                        **({"tag": "pd"} if o["unified_pd"] else {"tag": "pqk", "bufs": 1}),
                    )
                    for jq in range(4):
                        jt = half * 4 + jq
                        for it in range(4):
                            nc.tensor.matmul(
                                pqk[:, jq * 256 : (jq + 1) * 256],
                                lhsT=wq_sb[:, jt, it, :],
                                rhs=xt[:, it, :],
                                start=(it == 0),
                                stop=(it == 3),
                            )
                    if o["evac_chunks"] == 1:
                        nc.vector.tensor_copy(
                            out=qkT[:, half * 1024 : (half + 1) * 1024], in_=pqk[:]
                        )
                    else:
                        ch = 1024 // o["evac_chunks"]
                        for ci in range(o["evac_chunks"]):
                            nc.vector.tensor_copy(
                                out=qkT[
                                    :,
                                    half * 1024 + ci * ch : half * 1024
                                    + (ci + 1) * ch,
                                ],
                                in_=pqk[:, ci * ch : (ci + 1) * ch],
                            )

                # v projection (untransposed): v[n, j]
                for nt in range(2):
                    pv = pod.tile([128, 512], F32, tag="pod", name="pv")
                    for it in range(4):
                        nc.tensor.matmul(
                            pv[:],
                            lhsT=xt[:, it, nt * 128 : (nt + 1) * 128],
                            rhs=wq_sb[:, 8:12, it, :],
                            start=(it == 0),
                            stop=(it == 3),
                        )
                    if o["vt_on_act"]:
                        nc.scalar.copy(out=vt[:, nt, :], in_=pv[:])
                    else:
                        nc.vector.tensor_copy(out=vt[:, nt, :], in_=pv[:])

                # stage zero-padded q tiles (pure DMA, no engine cost);
                # head h = 4g+hp lives at partitions [32hp, 32hp+32),
                # dst slot h, src block g.
                qz = q0[b % o["q0_bufs"]]
                if o["q0_merged_dma"]:
                    qz_g = qz.rearrange("p (g q) n -> p g q n", q=4)
                    for hp in range(4):
                        nc.sync.dma_start(
                            out=qz_g[32 * hp : 32 * (hp + 1), :, hp, :],
                            in_=qkT[32 * hp : 32 * (hp + 1), 0:1024].rearrange(
                                "p (g n) -> p g n", n=256
                            ),
                        )
                else:
                    for h in range(H):
                        hp, g = h % 4, h // 4
                        nc.sync.dma_start(
                            out=qz[32 * hp : 32 * (hp + 1), h, :],
                            in_=qkT[32 * hp : 32 * (hp + 1), g * 256 : (g + 1) * 256],
                        )

                attn = [
                    attnpool.tile([128, 4096], BF16, tag=f"attn{mt}", name=f"attn{mt}")
                    for mt in range(2)
                ]
                return dict(xt=xt, qkT=qkT, vt=vt, qz=qz, attn=attn, b=b)

            def emit_dots_group(st, g, last=False):
                """Packed dots + exp + bias multiply for head-group g."""
                qkT, qz, attn = st["qkT"], st["qz"], st["attn"]
                for mt in range(2):
                    pd = pbig.tile([128, 1024], F32, tag="pd", name="pd")
                    for t in range(2):
                        nc.tensor.matmul(
                            pd[:, t * 512 : (t + 1) * 512],
                            lhsT=qkT[
                                :,
                                (4 + g) * 256 + mt * 128 : (4 + g) * 256
                                + (mt + 1) * 128,
                            ],
                            rhs=qz[:, 4 * g + 2 * t : 4 * g + 2 * t + 2, :],
                            start=True,
                            stop=True,
                        )
                    if o["exp_split"]:
                        # 512-col halves: exp(t0) starts as soon as the first
                        # dots matmul drains (bank-level dep), and the pd slot
                        # frees ~0.5us earlier for the next dots pair
                        for t in range(2):
                            lo = g * 1024 + t * 512
                            nc.scalar.activation(
                                out=attn[mt][:, lo : lo + 512],
                                in_=pd[:, t * 512 : (t + 1) * 512],
                                func=EXP,
                            )
                            eng = (
                                nc.vector
                                if last
                                or (4 * g + 2 * mt + t) % 16 < 2 * o["bias_dve_mod"]
                                else nc.gpsimd
                            )
                            eng.tensor_mul(
                                attn[mt][:, lo : lo + 512],
                                attn[mt][:, lo : lo + 512],
                                eb_sb[:, mt, lo : lo + 512],
                            )
                    else:
                        nc.scalar.activation(
                            out=attn[mt][:, g * 1024 : (g + 1) * 1024],
                            in_=pd[:],
                            func=EXP,
                        )
                        eng = (
                            nc.vector
                            if last or (2 * g + mt) % 8 < o["bias_dve_mod"]
                            else nc.gpsimd
                        )
                        eng.tensor_mul(
                            attn[mt][:, g * 1024 : (g + 1) * 1024],
                            attn[mt][:, g * 1024 : (g + 1) * 1024],
                            eb_sb[:, mt, g * 1024 : (g + 1) * 1024],
                        )

            def emit_attnv_group(st, g, outT):
                """attn@v + denominators + normalize for head-group g."""
                vt, attn = st["vt"], st["attn"]
                od = pod.tile([128, 512], F32, tag="pod", name="od")
                for mt in range(2):
                    bundle = []
                    for hp in range(4):
                        h = 4 * g + hp
                        bi = nc.tensor.matmul(
                            od[32 * hp : 32 * (hp + 1), 0:256],
                            lhsT=vt[:, mt, 32 * h : 32 * h + 32],
                            rhs=attn[mt][:, h * 256 : (h + 1) * 256],
                            start=(mt == 0),
                            stop=(mt == 1),
                            tile_position=(0, 32 * hp),
                            skip_group_check=True,
                        )
                        bundle.append(bi.ins.name)
                bundle = []
                for mt in range(2):
                    for hp in range(4):
                        h = 4 * g + hp
                        bi = nc.tensor.matmul(
                            od[32 * hp : 32 * (hp + 1), 256:512],
                            lhsT=ones32[:],
                            rhs=attn[mt][:, h * 256 : (h + 1) * 256],
                            start=(mt == 0),
                            stop=(mt == 1),
                            tile_position=(0, 32 * hp),
                            skip_group_check=True,
                        )
                        bundle.append(bi.ins.name)
                bundles.append(bundle)
                if o["stt_div"]:
                    nc.vector.tensor_tensor(
                        outT[:, g * 256 : (g + 1) * 256],
                        od[:, 0:256],
                        od[:, 256:512],
                        mybir.AluOpType.divide,
                    )
                else:
                    r = small.tile([128, 256], F32, tag="r")
                    nc.vector.reciprocal_approx_fast(out=r[:], in_=od[:, 256:512])
                    nc.vector.tensor_mul(
                        outT[:, g * 256 : (g + 1) * 256], od[:, 0:256], r[:]
                    )

            def emit_outproj(st, outT):
                b = st["b"]
                for nt in range(2):
                    py = pod.tile([128, 512], F32, tag="pod", name="py")
                    for ot in range(4):
                        nc.tensor.matmul(
                            py[:],
                            lhsT=outT[
                                :, ot * 256 + nt * 128 : ot * 256 + nt * 128 + 128
                            ],
                            rhs=w2_sb[:, ot, :],
                            start=(ot == 0),
                            stop=(ot == 3),
                        )
                    ysb = small.tile([128, 512], F32, tag="ysb", name="ysb")
                    # bias add rides the DVE evacuation (same cost as a copy);
                    # DVE, not ACT, keeps the strict-FIFO Scalar queue free
                    # for the exps that gate dots PSUM slots
                    nc.vector.tensor_add(ysb[:], py[:], bout_bc[:])
                    nc.sync.dma_start(out=y[b, nt], in_=ysb[:])

            def emit_back(st):
                outT = small.tile([128, 1024], BF16, tag="outT")
                for g in range(4):
                    emit_attnv_group(st, g, outT)
                emit_outproj(st, outT)

            if o["depth"] == 2 and o["interleave"]:
                # depth-2 pipeline: front(b) || dots(b-1) || attn@v(b-2)
                sts = []
                for rep in range(repeats):
                    for b in range(bl):
                        st = emit_front(rep, b)
                        outT = small.tile([128, 1024], BF16, tag="outT")
                        for g in range(4):
                            if sts:
                                emit_dots_group(sts[-1], g)
                            if len(sts) >= 2:
                                emit_attnv_group(sts[-2], g, outT)
                        if len(sts) >= 2:
                            emit_outproj(sts[-2], outT)
                        sts = sts[-1:] + [st]
                # drain: dots(last) + attn@v(second-last), then back(last)
                outT = small.tile([128, 1024], BF16, tag="outT")
                for g in range(4):
                    emit_dots_group(sts[-1], g, last=True)
                    if len(sts) >= 2:
                        emit_attnv_group(sts[-2], g, outT)
                if len(sts) >= 2:
                    emit_outproj(sts[-2], outT)
                emit_back(sts[-1])
            else:
                prev = None
                for rep in range(repeats):
                    for b in range(bl):
                        st = emit_front(rep, b)
                        if o["interleave"]:
                            outT = small.tile([128, 1024], BF16, tag="outT")
                            for g in range(4):
                                emit_dots_group(st, g)
                                if prev is not None:
                                    emit_attnv_group(prev, g, outT)
                            if prev is not None:
                                emit_outproj(prev, outT)
                        else:
                            for g in range(4):
                                emit_dots_group(st, g)
                            if prev is not None:
                                emit_back(prev)
                        prev = st
                if o["depth"] == 2 and not o["interleave"]:
                    pass
                emit_back(prev)

    if o["hoist"]:
        _hoist_ldw_bundles(nc, bundles)
    nc.compile()
    return nc


def _get_nc(bl: int, repeats: int = 1, opts: dict | None = None):
    key = (bl, repeats, tuple(sorted((opts or {}).items())))
    if key not in _CACHE:
        _CACHE[key] = _build(bl, repeats, opts)
    return _CACHE[key]


def _prep_inputs(x, w_qkv, rel_bias_table, w_out, b_out):
    """Host-side layout prep: transpose/tile/bf16-cast, bias-table gather."""
    x = np.asarray(x, np.float32)
    w_qkv = np.asarray(w_qkv, np.float32).copy()
    rel_bias_table = np.asarray(rel_bias_table, np.float32)
    w_out = np.asarray(w_out, np.float32)
    b_out = np.asarray(b_out, np.float32)

    # fold the attention scale into the q columns of w_qkv
    w_qkv[:, :OUP] *= SCALE

    # xT_dev[b, p, it, n] = x[b, n, it*128+p]
    xT = np.ascontiguousarray(
        x.transpose(0, 2, 1).reshape(B, 4, 128, N).transpose(0, 2, 1, 3)
    ).astype(NPBF16)
    # wqkv_dev[p, jt, it, jj] = w_qkv[it*128+p, jt*128+jj]
    wqkv_dev = np.ascontiguousarray(
        w_qkv.reshape(4, 128, 12, 128).transpose(1, 2, 0, 3)
    ).astype(NPBF16)
    # w2t_dev[p, ot, q] = w_out.T[ot*128+p, q] = w_out[q, ot*128+p]
    w2t_dev = np.ascontiguousarray(
        w_out.T.reshape(4, 128, OUP).transpose(1, 0, 2)
    ).astype(NPBF16)
    # bias[n, m, h]; ebT_dev[p, mt, h*256+n] = exp(bias[n, mt*128+p, h])
    rel_idx = _relative_index(16, 16)
    bias = rel_bias_table[rel_idx].reshape(N, N, H)  # [n, m, h]
    ebT = np.exp(bias.transpose(2, 1, 0))  # [h, m, n]
    ebT_dev = np.ascontiguousarray(
        ebT.reshape(H, 2, 128, N).transpose(2, 1, 0, 3).reshape(128, 2, H * N)
    ).astype(NPBF16)
    bout_dev = b_out.reshape(1, OUP).astype(np.float32)
    return xT, wqkv_dev, w2t_dev, ebT_dev, bout_dev


def kernel(x, w_qkv, rel_bias_table, w_out, b_out, ih, iw):
    assert int(ih) == 16 and int(iw) == 16
    xT, wqkv_dev, w2t_dev, ebT_dev, bout_dev = _prep_inputs(
        x, w_qkv, rel_bias_table, w_out, b_out
    )

    nc = _get_nc(BL)
    in_maps = []
    for c in range(NCORES):
        in_maps.append(
            {
                "xT": np.ascontiguousarray(xT[c * BL : (c + 1) * BL]),
                "wqkv": wqkv_dev,
                "w2t": w2t_dev,
                "ebT": ebT_dev,
                "bout": bout_dev,
                "boutb": bout_dev.astype(NPBF16),
            }
        )

    trace = bool(os.environ.get("BASS_TRACE_KERNEL"))
    if trace:
        try:
            from antenv.axon_hooks import get_axon_ntff_profile_hook  # noqa: F401
        except ImportError:
            trace = False
    res = run_bass_kernel_spmd(nc, in_maps, core_ids=list(range(NCORES)), trace=trace)
    kernel.last_result = res
    if res.exec_time_ns is not None:
        print(f"HW exec time: {res.exec_time_ns} ns")

    y = np.concatenate(
        [r["y"].reshape(BL, N, OUP) for r in res.results], axis=0
    ).astype(np.float32)
    return y


kernel.last_result = None
